# Initial kernel scaffold
#
"""Trainium2 Bass kernel for nn_CrossLayerLight (cross-cloud KNN message passing).

Sharding: 8 cores = 2 directions x 2 batches x 2 query-halves.
Each core: 4096 queries vs 8192 candidates.

Per-core device pipeline:
  A) v-table build: v[j] = feat2[j] + xyz2[j] @ pos_w^T   (row-major, split to
     bf16 hi/lo pair rows [8192, 128] in DRAM for gathering)
  B) u-table build: u[q] = feat1[q] - xyz1[q] @ pos_w^T + pos_b
     (feature-major, bf16 hi/lo stacked [128, 4096] in SBUF)
  C) per 128-query tile:
     - scores = 2 q.p - |p|^2 via 30-row bf16 3-term-split matmul (PE, exact to ~1e-6)
     - exact top-16 via DVE max8 / max_index / match_replace
     - idx transpose+replicate via PE transpose -> dma_gather of v-pairs
     - z0 = v_hi + v_lo + u (PE identity matmuls), leaky (ACT)
     - 2x 64x64 MLP (PE) + leaky, max-pool over k (DVE), final 64->128 linear
       with fused bias (PE), DMA out.
"""

import sys
import os
import numpy as np
import ml_dtypes

sys.path.insert(0, "/opt/trn_rl_repo")

import concourse.bacc as bacc  # noqa: E402
import concourse.mybir as mybir  # noqa: E402
from concourse.bass_utils import run_bass_kernel_spmd  # noqa: E402
from concourse.tile import TileContext  # noqa: E402

BF16 = ml_dtypes.bfloat16
F32 = mybir.dt.float32
BF = mybir.dt.bfloat16
U16 = mybir.dt.uint16
I16 = mybir.dt.int16

NQ_TOT = 4096   # queries per core
NCAND = 8192    # candidates per core
D = 64          # feature dim
KNN = 16
NROW = 30       # score matmul contraction rows
LEAKY = 0.1

_CACHE = {}


def _leaky_act(nc, out, in_, bias=0.0, nbias=0.0, sim_compat=False, pool=None):
    """out = leaky(in_ + bias); bias is float or [P,1] AP; nbias = -bias."""
    if not sim_compat:
        nc.scalar.activation(out, in_, mybir.ActivationFunctionType.Lrelu,
                             bias=bias, alpha=LEAKY)
        return
    # sim-compatible: leaky(x+b) = relu(x+b) - 0.1*relu(-x-b)
    p = pool.tile(list(out.shape), F32, tag="lk_a")
    n = pool.tile(list(out.shape), F32, tag="lk_b")
    nc.scalar.activation(p[:], in_, mybir.ActivationFunctionType.Relu, bias=bias)
    nc.scalar.activation(n[:], in_, mybir.ActivationFunctionType.Relu,
                         bias=nbias, scale=-1.0)
    nc.vector.tensor_scalar(n[:], n[:], -LEAKY, scalar2=None,
                            op0=mybir.AluOpType.mult)
    nc.vector.tensor_add(out, p[:], n[:])


def build_nc(nq_tot=NQ_TOT, ncand=NCAND, sim_compat=False, use_gather=True,
             c_repeats=1):
    nc = bacc.Bacc()
    ntiles = nq_tot // 128

    # ---- external inputs ----
    sc_lhsT = nc.dram_tensor("sc_lhsT", [NROW, nq_tot], BF, kind="ExternalInput")
    sc_rhs = nc.dram_tensor("sc_rhs", [NROW, ncand], BF, kind="ExternalInput")
    vb_lhsT = nc.dram_tensor("vb_lhsT", [67, ncand], F32, kind="ExternalInput")
    vb_rhs = nc.dram_tensor("vb_rhs", [67, D], F32, kind="ExternalInput")
    ub_lhsT = nc.dram_tensor("ub_lhsT", [68, D], F32, kind="ExternalInput")
    ub_rhs = nc.dram_tensor("ub_rhs", [68, nq_tot], F32, kind="ExternalInput")
    F32R = mybir.dt.float32r
    w0T = nc.dram_tensor("w0T", [2 * D, D], F32R, kind="ExternalInput")
    w1T = nc.dram_tensor("w1T", [2 * D, D], F32R, kind="ExternalInput")
    b0c = nc.dram_tensor("b0c", [D, 1], F32, kind="ExternalInput")
    b1c = nc.dram_tensor("b1c", [D, 1], F32, kind="ExternalInput")
    b0n = nc.dram_tensor("b0n", [D, 1], F32, kind="ExternalInput")
    b1n = nc.dram_tensor("b1n", [D, 1], F32, kind="ExternalInput")
    t_rhs = nc.dram_tensor("t_rhs", [2 * D, 128], F32R, kind="ExternalInput")
    tb_row = nc.dram_tensor("tb_row", [1, 128], F32, kind="ExternalInput")
    ii128 = nc.dram_tensor("ii128", [128, D], BF, kind="ExternalInput")
    id128u = nc.dram_tensor("id128u", [128, 128], F32, kind="ExternalInput")

    out = nc.dram_tensor("out", [nq_tot, 128], F32, kind="ExternalOutput")

    with TileContext(nc) as tc:
        with (
            tc.tile_pool(name="const", bufs=1) as cst,
            tc.tile_pool(name="dram", bufs=1, space="DRAM") as dram,
        ):
            vpair = dram.tile([ncand, 128], BF)

            # persistent SBUF tiles
            sc_l = cst.tile([NROW, nq_tot], BF)
            sc_r = cst.tile([NROW, ncand], BF)
            uhl = cst.tile([128, nq_tot], BF)      # rows 0:64 u_hi, 64:128 u_lo
            w0s = cst.tile([2 * D, D], F32R)
            w1s = cst.tile([2 * D, D], F32R)
            b0s = cst.tile([D, 1], F32)
            b1s = cst.tile([D, 1], F32)
            b0ns = cst.tile([D, 1], F32)
            b1ns = cst.tile([D, 1], F32)
            trs = cst.tile([2 * D, 128], F32R)
            tbs = cst.tile([1, 128], F32)
            ones1 = cst.tile([1, 128], F32)
            iis = cst.tile([128, D], BF)
            idu = cst.tile([128, 128], F32)
            for dst, src in [(sc_l, sc_lhsT), (sc_r, sc_rhs), (w0s, w0T),
                             (w1s, w1T), (b0s, b0c), (b1s, b1c), (b0ns, b0n),
                             (b1ns, b1n), (trs, t_rhs), (tbs, tb_row),
                             (iis, ii128), (idu, id128u)]:
                nc.sync.dma_start(out=dst[:], in_=src[:])
            nc.vector.memset(ones1[:], 1.0)

            # ---- phase A: v table ----
            with (
                tc.tile_pool(name="phA", bufs=2) as pha,
                tc.tile_pool(name="phA_ps", bufs=2, space="PSUM") as phaps,
            ):
                vbw = pha.tile([67, D], F32, tag="vbw")
                nc.sync.dma_start(out=vbw[:], in_=vb_rhs[:])
                njt = ncand // 128
                grp = 8  # j-tiles per psum fill
                for g in range(njt // grp):
                    pv = phaps.tile([128, grp * D], F32, tag="pv")
                    for s in range(grp):
                        jt = g * grp + s
                        vbl = pha.tile([67, 128], F32, tag="vbl")
                        nc.sync.dma_start(out=vbl[:], in_=vb_lhsT[:, jt * 128:(jt + 1) * 128])
                        nc.tensor.matmul(pv[:, s * D:(s + 1) * D], vbl[:], vbw[:],
                                         start=True, stop=True)
                    vhi = pha.tile([128, grp * D], BF, tag="vhi")
                    vlo = pha.tile([128, grp * D], BF, tag="vlo")
                    nc.scalar.activation(vhi[:], pv[:], mybir.ActivationFunctionType.Copy)
                    nc.vector.tensor_sub(vlo[:], pv[:], vhi[:])
                    # vpair rows j = g*grp*128 + s*128 + p ; hi cols 0:64, lo 64:128
                    dst = vpair[g * grp * 128:(g + 1) * grp * 128, :]
                    dst_hi = dst[:, 0:D].rearrange("(s p) f -> p s f", p=128)
                    dst_lo = dst[:, D:128].rearrange("(s p) f -> p s f", p=128)
                    nc.sync.dma_start(out=dst_hi, in_=vhi[:].rearrange("p (s f) -> p s f", f=D))
                    nc.sync.dma_start(out=dst_lo, in_=vlo[:].rearrange("p (s f) -> p s f", f=D))

            # ---- phase B: u table ----
            with (
                tc.tile_pool(name="phB", bufs=2) as phb,
                tc.tile_pool(name="phB_ps", bufs=2, space="PSUM") as phbps,
            ):
                ubw = phb.tile([68, D], F32, tag="ubw")
                nc.sync.dma_start(out=ubw[:], in_=ub_lhsT[:])
                uchunk = min(2048, nq_tot)
                for h in range(nq_tot // uchunk):
                    ur = phb.tile([68, uchunk], F32, tag="ur")
                    nc.sync.dma_start(out=ur[:], in_=ub_rhs[:, h * uchunk:(h + 1) * uchunk])
                    pu = phbps.tile([D, uchunk], F32, tag="pu")
                    for j in range(uchunk // 512 or 1):
                        w = min(512, uchunk)
                        nc.tensor.matmul(pu[:, j * w:(j + 1) * w], ubw[:],
                                         ur[:, j * w:(j + 1) * w], start=True, stop=True)
                    nc.scalar.activation(uhl[0:D, h * uchunk:(h + 1) * uchunk], pu[:],
                                         mybir.ActivationFunctionType.Copy)
                    nc.vector.tensor_sub(uhl[D:128, h * uchunk:(h + 1) * uchunk], pu[:],
                                         uhl[0:D, h * uchunk:(h + 1) * uchunk])

            # ---- phase C: per-tile ----
            with (
                tc.tile_pool(name="sc", bufs=2) as scp,
                tc.tile_pool(name="wk", bufs=2) as wk,
                tc.tile_pool(name="ps_sc", bufs=2, space="PSUM") as pssc,
                tc.tile_pool(name="ps_z", bufs=2, space="PSUM") as psz,
                tc.tile_pool(name="ps_tr", bufs=1, space="PSUM") as pstr,
                tc.tile_pool(name="ps_t1", bufs=1, space="PSUM") as pst1,
            ):
                for qt in range(ntiles * c_repeats):
                    qt = qt % ntiles
                    q0 = qt * 128
                    # C1: scores
                    ssb = scp.tile([128, ncand], F32, tag="ssb")
                    for h in range(ncand // 1024):
                        pst = pssc.tile([128, 1024], F32, tag="psc")
                        for j in range(2):
                            c0 = h * 1024 + j * 512
                            nc.tensor.matmul(pst[:, j * 512:(j + 1) * 512],
                                             sc_l[:, q0:q0 + 128],
                                             sc_r[:, c0:c0 + 512],
                                             start=True, stop=True)
                        nc.scalar.activation(ssb[:, h * 1024:(h + 1) * 1024], pst[:],
                                             mybir.ActivationFunctionType.Copy)
                    # C2: top16
                    v8a = wk.tile([128, 8], F32, tag="v8a")
                    v8b = wk.tile([128, 8], F32, tag="v8b")
                    i32 = wk.tile([128, 16], mybir.dt.uint32, tag="i32")
                    nc.vector.max(out=v8a[:], in_=ssb[:])
                    nc.vector.max_index(out=i32[:, 0:8], in_max=v8a[:], in_values=ssb[:])
                    nc.vector.match_replace(out=ssb[:], in_to_replace=v8a[:],
                                            in_values=ssb[:], imm_value=-1e30)
                    nc.vector.max(out=v8b[:], in_=ssb[:])
                    nc.vector.max_index(out=i32[:, 8:16], in_max=v8b[:], in_values=ssb[:])
                    # C3: idx -> fp32 -> replicate x8 in free dim -> PE transpose
                    i16f = wk.tile([128, 16], F32, tag="i16f")
                    nc.vector.tensor_copy(i16f[:], i32[:])
                    i16r = wk.tile([128, 128], F32, tag="i16r")
                    rep = i16f[:].unsqueeze(1).to_broadcast([128, 8, 16])
                    nc.vector.tensor_copy(i16r[:].rearrange("p (r k) -> p r k", k=16), rep)
                    ptr = pstr.tile([128, 128], F32, tag="ptr")
                    nc.tensor.transpose(ptr[:], i16r[:], idu[:])
                    idxs = wk.tile([128, 128], I16, tag="idxs")
                    nc.vector.tensor_copy(idxs[:], ptr[:])
                    # C4: gather v pairs -> [128, 2048] bf16 (cols q*16+k)
                    gt = wk.tile([128, 1, 2048], BF, tag="gt")
                    if use_gather:
                        nc.gpsimd.dma_gather(out_ap=gt[:], in_ap=vpair[:], idxs_ap=idxs[:],
                                             num_idxs=2048, num_idxs_reg=2048,
                                             elem_size=128, transpose=True,
                                             single_packet=False)
                    else:
                        nc.vector.memset(gt[:], 0.0)
                    gtf = gt[:].rearrange("p a n -> p (a n)")
                    # C5-C8 per 512-col block (32 queries)
                    # leaky(y) folded into next matmul: rhs = [relu(y); relu(-y)],
                    # lhsT = [W^T; -0.1 W^T].  Final leaky commutes with max-pool.
                    pooled = wk.tile([D, 128], F32, tag="pooled")
                    # materialize u broadcast (each query's u column repeated 16x)
                    urep = wk.tile([128, 2048], BF, tag="urep")
                    ub_b = uhl[:, q0:q0 + 128].unsqueeze(2).to_broadcast([128, 128, KNN])
                    nc.scalar.activation(urep[:].rearrange("p (q k) -> p q k", k=KNN),
                                         ub_b, mybir.ActivationFunctionType.Copy)
                    for cb in range(4):
                        cbase = cb * 512
                        pz0 = psz.tile([D, 512], F32, tag="pz")
                        nc.tensor.matmul(pz0[:], iis[:], gtf[:, cbase:cbase + 512],
                                         start=True, stop=False)
                        nc.tensor.matmul(pz0[:], iis[:], urep[:, cbase:cbase + 512],
                                         start=False, stop=True)
                        rp0 = wk.tile([2 * D, 512], F32R, tag="rp0")
                        nc.scalar.activation(rp0[0:D, :], pz0[:],
                                             mybir.ActivationFunctionType.Relu)
                        nc.scalar.activation(rp0[D:2 * D, :], pz0[:],
                                             mybir.ActivationFunctionType.Relu, scale=-1.0)
                        pz1 = psz.tile([D, 512], F32, tag="pz")
                        nc.tensor.matmul(pz1[:], w0s[:], rp0[:], start=True, stop=True)
                        rp1 = wk.tile([2 * D, 512], F32R, tag="rp0")
                        nc.scalar.activation(rp1[0:D, :], pz1[:],
                                             mybir.ActivationFunctionType.Relu, bias=b0s[:])
                        nc.scalar.activation(rp1[D:2 * D, :], pz1[:],
                                             mybir.ActivationFunctionType.Relu,
                                             bias=b0ns[:], scale=-1.0)
                        pz2 = psz.tile([D, 512], F32, tag="pz")
                        nc.tensor.matmul(pz2[:], w1s[:], rp1[:], start=True, stop=True)
                        # pool over k=16 straight from PSUM (pre-activation; leaky
                        # and +b1 are applied after pooling - both monotonic)
                        nc.vector.tensor_reduce(
                            out=pooled[:, cb * 32:(cb + 1) * 32],
                            in_=pz2[:].rearrange("p (q k) -> p q k", k=KNN),
                            axis=mybir.AxisListType.X, op=mybir.AluOpType.max)
                    # t-linear: lhsT = [relu(pooled+b1); relu(-pooled-b1)] (f32r),
                    # rhs = [tw^T; -0.1 tw^T]; bias via K=1 ones x tb matmul.
                    tl = wk.tile([2 * D, 128], F32R, tag="tl")
                    nc.scalar.activation(tl[0:D, :], pooled[:],
                                         mybir.ActivationFunctionType.Relu, bias=b1s[:])
                    nc.scalar.activation(tl[D:2 * D, :], pooled[:],
                                         mybir.ActivationFunctionType.Relu,
                                         bias=b1ns[:], scale=-1.0)
                    pt1 = pst1.tile([128, 128], F32, tag="pt1")
                    nc.tensor.matmul(pt1[:], tl[:], trs[:], start=True, stop=False)
                    nc.tensor.matmul(pt1[:], ones1[:], tbs[:], start=False, stop=True)
                    outt = wk.tile([128, 128], F32, tag="outt")
                    nc.scalar.activation(outt[:], pt1[:], mybir.ActivationFunctionType.Copy)
                    nc.sync.dma_start(out=out[q0:q0 + 128, :], in_=outt[:])

    nc.compile()
    return nc


def _split_bf16(x, n):
    parts = []
    rem = np.asarray(x, np.float64)
    for _ in range(n):
        p = rem.astype(BF16)
        parts.append(p)
        rem = rem - p.astype(np.float64)
    return parts


def prep_core_inputs(qxyz, qfeat, cxyz, cfeat, pos_w, pos_b, tw, tb):
    """Build the per-core input map. All host work is O(N*small) layout prep."""
    nq = qxyz.shape[0]
    ncand = cxyz.shape[0]
    A = _split_bf16(2.0 * qxyz, 3)           # each [nq, 3]
    P = _split_bf16(cxyz, 3)                 # each [ncand, 3]
    m = _split_bf16(-np.sum(cxyz.astype(np.float64) ** 2, -1), 3)

    # order products by (i+j) descending so small terms accumulate first
    rows_q = []
    rows_c = []
    prods = sorted(((i, j) for i in range(3) for j in range(3)),
                   key=lambda t: -(t[0] + t[1]))
    for (i, j) in prods:
        for c in range(3):
            rows_q.append(A[i][:, c])
            rows_c.append(P[j][:, c])
    ones = np.ones(nq, BF16)
    for t in (m[2], m[1], m[0]):
        rows_q.append(ones)
        rows_c.append(t)
    sc_lhsT = np.stack(rows_q).astype(BF16)      # [30, nq]
    sc_rhs = np.stack(rows_c).astype(BF16)       # [30, ncand]

    vb_lhsT = np.concatenate([cxyz.T, cfeat.T]).astype(np.float32)       # [67, ncand]
    vb_rhs = np.concatenate([pos_w.T, np.eye(D)]).astype(np.float32)     # [67, 64]
    ub_lhsT = np.concatenate([-pos_w.T, np.eye(D), pos_b[None, :]]).astype(np.float32)  # [68, 64]
    ub_rhs = np.concatenate([qxyz.T, qfeat.T, np.ones((1, nq))]).astype(np.float32)     # [68, nq]

    t_rhs = np.concatenate([tw.T, -LEAKY * tw.T]).astype(np.float32)     # [128, 128]
    tb_row = tb[None, :].astype(np.float32)
    ii = np.concatenate([np.eye(D), np.eye(D)]).astype(BF16)             # [128, 64]
    idu = np.eye(128).astype(np.float32)

    return {
        "sc_lhsT": sc_lhsT, "sc_rhs": sc_rhs,
        "vb_lhsT": vb_lhsT, "vb_rhs": vb_rhs,
        "ub_lhsT": ub_lhsT, "ub_rhs": ub_rhs,
        "w0T": None, "w1T": None,  # filled by caller (shared)
        "b0c": None, "b1c": None, "b0n": None, "b1n": None,
        "t_rhs": t_rhs, "tb_row": tb_row, "ii128": ii, "id128u": idu,
    }


def build_in_maps(inputs):
    pc1 = np.asarray(inputs["pc1"]); pc2 = np.asarray(inputs["pc2"])
    feat1 = np.asarray(inputs["feat1"]); feat2 = np.asarray(inputs["feat2"])
    pos_w = np.asarray(inputs["pos_w"]); pos_b = np.asarray(inputs["pos_b"])
    w0 = np.asarray(inputs["mlp_w0"]); b0 = np.asarray(inputs["mlp_b0"])
    w1 = np.asarray(inputs["mlp_w1"]); b1 = np.asarray(inputs["mlp_b1"])
    t1w = np.asarray(inputs["t1_w"]); t1b = np.asarray(inputs["t1_b"])
    t2w = np.asarray(inputs["t2_w"]); t2b = np.asarray(inputs["t2_b"])

    w0T = np.concatenate([w0.T, -LEAKY * w0.T]).astype(np.float32)
    w1T = np.concatenate([w1.T, -LEAKY * w1.T]).astype(np.float32)
    b0c = b0.astype(np.float32)[:, None].copy()
    b1c = b1.astype(np.float32)[:, None].copy()

    half = NQ_TOT
    in_maps = []
    core_meta = []
    for d in range(2):
        for b in range(2):
            for h in range(2):
                if d == 0:
                    q, p, fq, fp, tw, tb = pc1[b], pc2[b], feat1[b], feat2[b], t1w, t1b
                else:
                    q, p, fq, fp, tw, tb = pc2[b], pc1[b], feat2[b], feat1[b], t2w, t2b
                sl = slice(h * half, (h + 1) * half)
                m = prep_core_inputs(q[sl], fq[sl], p, fp, pos_w, pos_b, tw, tb)
                m["w0T"] = w0T; m["w1T"] = w1T; m["b0c"] = b0c; m["b1c"] = b1c
                m["b0n"] = -b0c; m["b1n"] = -b1c
                in_maps.append(m)
                core_meta.append((d, b, h))
    return in_maps, core_meta


def kernel(pc1, pc2, feat1, feat2, pos_w, pos_b, mlp_w0, mlp_b0,
           mlp_w1, mlp_b1, t1_w, t1_b, t2_w, t2_b, _trace=False):
    pc1 = np.asarray(pc1)

    if "nc" not in _CACHE:
        _CACHE["nc"] = build_nc()
    nc = _CACHE["nc"]

    inputs = dict(pc1=pc1, pc2=pc2, feat1=feat1, feat2=feat2, pos_w=pos_w,
                  pos_b=pos_b, mlp_w0=mlp_w0, mlp_b0=mlp_b0, mlp_w1=mlp_w1,
                  mlp_b1=mlp_b1, t1_w=t1_w, t1_b=t1_b, t2_w=t2_w, t2_b=t2_b)
    in_maps, core_meta = build_in_maps(inputs)
    _CACHE["last_in_maps"] = in_maps

    res = run_bass_kernel_spmd(nc, in_maps, core_ids=list(range(8)), trace=_trace)
    _CACHE["last_res"] = res
    half = NQ_TOT

    B, N = pc1.shape[0], pc1.shape[1]
    f1 = np.zeros((B, N, 128), np.float32)
    f2 = np.zeros((B, N, 128), np.float32)
    for (dd, b, h), r in zip(core_meta, res.results):
        o = r["out"]
        tgt = f1 if dd == 0 else f2
        tgt[b, h * half:(h + 1) * half, :] = o
    return f1, f2


if __name__ == "__main__":
    # quick smoke with random data
    rng = np.random.default_rng(0)
    B, N = 2, 8192
    ins = {
        "pc1": rng.standard_normal((B, N, 3), np.float32),
        "pc2": rng.standard_normal((B, N, 3), np.float32),
        "feat1": rng.standard_normal((B, N, D), np.float32),
        "feat2": rng.standard_normal((B, N, D), np.float32),
        "pos_w": (rng.standard_normal((D, 3)) * 0.1).astype(np.float32),
        "pos_b": (rng.standard_normal((D,)) * 0.1).astype(np.float32),
        "mlp_w0": (rng.standard_normal((D, D)) * 0.1).astype(np.float32),
        "mlp_b0": (rng.standard_normal((D,)) * 0.1).astype(np.float32),
        "mlp_w1": (rng.standard_normal((D, D)) * 0.1).astype(np.float32),
        "mlp_b1": (rng.standard_normal((D,)) * 0.1).astype(np.float32),
        "t1_w": (rng.standard_normal((128, D)) * 0.1).astype(np.float32),
        "t1_b": (rng.standard_normal((128,)) * 0.1).astype(np.float32),
        "t2_w": (rng.standard_normal((128, D)) * 0.1).astype(np.float32),
        "t2_b": (rng.standard_normal((128,)) * 0.1).astype(np.float32),
    }
    f1, f2 = kernel(**ins)
    print("f1", f1.shape, "f2", f2.shape)



# revision 1
# speedup vs baseline: 4.6801x; 4.6801x over previous
"""Trainium2 Bass kernel for nn_CrossLayerLight (cross-cloud KNN message passing).

Sharding: 8 cores = 2 directions x 2 batches x 2 query-halves.
Each core: 4096 queries vs 8192 candidates.

Per-core device pipeline:
  A) v-table build: v[j] = feat2[j] + xyz2[j] @ pos_w^T   (row-major, split to
     bf16 hi/lo pair rows [8192, 128] in DRAM for gathering)
  B) u-table build: u[q] = feat1[q] - xyz1[q] @ pos_w^T + pos_b
     (feature-major, bf16 hi/lo stacked [128, 4096] in SBUF)
  C) per 128-query tile:
     - scores = 2 q.p - |p|^2 via 30-row bf16 3-term-split matmul (PE, exact to ~1e-6)
     - exact top-16 via DVE max8 / max_index / match_replace
     - idx transpose+replicate via PE transpose -> dma_gather of v-pairs
     - z0 = v_hi + v_lo + u (PE identity matmuls), leaky (ACT)
     - 2x 64x64 MLP (PE) + leaky, max-pool over k (DVE), final 64->128 linear
       with fused bias (PE), DMA out.
"""

import sys
import os
import numpy as np
import ml_dtypes

sys.path.insert(0, "/opt/trn_rl_repo")

import concourse.bacc as bacc  # noqa: E402
import concourse.mybir as mybir  # noqa: E402
from concourse.bass_utils import run_bass_kernel_spmd  # noqa: E402
from concourse.tile import TileContext  # noqa: E402

BF16 = ml_dtypes.bfloat16
F32 = mybir.dt.float32
BF = mybir.dt.bfloat16
U16 = mybir.dt.uint16
I16 = mybir.dt.int16

NQ_TOT = 4096   # queries per core
NCAND = 8192    # candidates per core
D = 64          # feature dim
KNN = 16
NROW = 30       # score matmul contraction rows
LEAKY = 0.1

_CACHE = {}


def _leaky_act(nc, out, in_, bias=0.0, nbias=0.0, sim_compat=False, pool=None):
    """out = leaky(in_ + bias); bias is float or [P,1] AP; nbias = -bias."""
    if not sim_compat:
        nc.scalar.activation(out, in_, mybir.ActivationFunctionType.Lrelu,
                             bias=bias, alpha=LEAKY)
        return
    # sim-compatible: leaky(x+b) = relu(x+b) - 0.1*relu(-x-b)
    p = pool.tile(list(out.shape), F32, tag="lk_a")
    n = pool.tile(list(out.shape), F32, tag="lk_b")
    nc.scalar.activation(p[:], in_, mybir.ActivationFunctionType.Relu, bias=bias)
    nc.scalar.activation(n[:], in_, mybir.ActivationFunctionType.Relu,
                         bias=nbias, scale=-1.0)
    nc.vector.tensor_scalar(n[:], n[:], -LEAKY, scalar2=None,
                            op0=mybir.AluOpType.mult)
    nc.vector.tensor_add(out, p[:], n[:])


def build_nc(nq_tot=NQ_TOT, ncand=NCAND, sim_compat=False, use_gather=True,
             c_repeats=1):
    nc = bacc.Bacc()
    ntiles = nq_tot // 128

    # ---- external inputs ----
    sc_lhsT = nc.dram_tensor("sc_lhsT", [NROW, nq_tot], BF, kind="ExternalInput")
    sc_rhs = nc.dram_tensor("sc_rhs", [NROW, ncand], BF, kind="ExternalInput")
    vb_lhsT = nc.dram_tensor("vb_lhsT", [67, ncand], F32, kind="ExternalInput")
    vb_rhs = nc.dram_tensor("vb_rhs", [67, D], F32, kind="ExternalInput")
    ub_lhsT = nc.dram_tensor("ub_lhsT", [68, D], F32, kind="ExternalInput")
    ub_rhs = nc.dram_tensor("ub_rhs", [68, nq_tot], F32, kind="ExternalInput")
    F32R = mybir.dt.float32r
    w0T = nc.dram_tensor("w0T", [2 * D, D], F32R, kind="ExternalInput")
    w1T = nc.dram_tensor("w1T", [2 * D, D], F32R, kind="ExternalInput")
    b0c = nc.dram_tensor("b0c", [D, 1], F32, kind="ExternalInput")
    b1c = nc.dram_tensor("b1c", [D, 1], F32, kind="ExternalInput")
    b0n = nc.dram_tensor("b0n", [D, 1], F32, kind="ExternalInput")
    b1n = nc.dram_tensor("b1n", [D, 1], F32, kind="ExternalInput")
    t_rhs = nc.dram_tensor("t_rhs", [2 * D, 128], F32R, kind="ExternalInput")
    tb_row = nc.dram_tensor("tb_row", [1, 128], F32, kind="ExternalInput")
    ii128 = nc.dram_tensor("ii128", [128, D], BF, kind="ExternalInput")
    id128u = nc.dram_tensor("id128u", [128, 128], F32, kind="ExternalInput")

    out = nc.dram_tensor("out", [nq_tot, 128], F32, kind="ExternalOutput")

    with TileContext(nc) as tc:
        with (
            tc.tile_pool(name="const", bufs=1) as cst,
            tc.tile_pool(name="dram", bufs=1, space="DRAM") as dram,
        ):
            vpair = dram.tile([ncand, 128], BF)

            # persistent SBUF tiles
            sc_l = cst.tile([NROW, nq_tot], BF)
            sc_r = cst.tile([NROW, ncand], BF)
            uhl = cst.tile([128, nq_tot], BF)      # rows 0:64 u_hi, 64:128 u_lo
            w0s = cst.tile([2 * D, D], F32R)
            w1s = cst.tile([2 * D, D], F32R)
            b0s = cst.tile([D, 1], F32)
            b1s = cst.tile([D, 1], F32)
            b0ns = cst.tile([D, 1], F32)
            b1ns = cst.tile([D, 1], F32)
            trs = cst.tile([2 * D, 128], F32R)
            tbs = cst.tile([1, 128], F32)
            ones1 = cst.tile([1, 128], F32)
            iis = cst.tile([128, D], BF)
            idu = cst.tile([128, 128], F32)
            for dst, src in [(sc_l, sc_lhsT), (sc_r, sc_rhs), (w0s, w0T),
                             (w1s, w1T), (b0s, b0c), (b1s, b1c), (b0ns, b0n),
                             (b1ns, b1n), (trs, t_rhs), (tbs, tb_row),
                             (iis, ii128), (idu, id128u)]:
                nc.sync.dma_start(out=dst[:], in_=src[:])
            nc.vector.memset(ones1[:], 1.0)

            # ---- phase A: v table ----
            with (
                tc.tile_pool(name="phA", bufs=2) as pha,
                tc.tile_pool(name="phA_ps", bufs=2, space="PSUM") as phaps,
            ):
                vbw = pha.tile([67, D], F32, tag="vbw")
                nc.sync.dma_start(out=vbw[:], in_=vb_rhs[:])
                njt = ncand // 128
                grp = 8  # j-tiles per psum fill
                for g in range(njt // grp):
                    pv = phaps.tile([128, grp * D], F32, tag="pv")
                    for s in range(grp):
                        jt = g * grp + s
                        vbl = pha.tile([67, 128], F32, tag="vbl")
                        nc.sync.dma_start(out=vbl[:], in_=vb_lhsT[:, jt * 128:(jt + 1) * 128])
                        nc.tensor.matmul(pv[:, s * D:(s + 1) * D], vbl[:], vbw[:],
                                         start=True, stop=True)
                    vhi = pha.tile([128, grp * D], BF, tag="vhi")
                    vlo = pha.tile([128, grp * D], BF, tag="vlo")
                    nc.scalar.activation(vhi[:], pv[:], mybir.ActivationFunctionType.Copy)
                    nc.vector.tensor_sub(vlo[:], pv[:], vhi[:])
                    # vpair rows j = g*grp*128 + s*128 + p ; hi cols 0:64, lo 64:128
                    dst = vpair[g * grp * 128:(g + 1) * grp * 128, :]
                    dst_hi = dst[:, 0:D].rearrange("(s p) f -> p s f", p=128)
                    dst_lo = dst[:, D:128].rearrange("(s p) f -> p s f", p=128)
                    nc.sync.dma_start(out=dst_hi, in_=vhi[:].rearrange("p (s f) -> p s f", f=D))
                    nc.sync.dma_start(out=dst_lo, in_=vlo[:].rearrange("p (s f) -> p s f", f=D))

            # ---- phase B: u table ----
            with (
                tc.tile_pool(name="phB", bufs=2) as phb,
                tc.tile_pool(name="phB_ps", bufs=2, space="PSUM") as phbps,
            ):
                ubw = phb.tile([68, D], F32, tag="ubw")
                nc.sync.dma_start(out=ubw[:], in_=ub_lhsT[:])
                uchunk = min(2048, nq_tot)
                for h in range(nq_tot // uchunk):
                    ur = phb.tile([68, uchunk], F32, tag="ur")
                    nc.sync.dma_start(out=ur[:], in_=ub_rhs[:, h * uchunk:(h + 1) * uchunk])
                    pu = phbps.tile([D, uchunk], F32, tag="pu")
                    for j in range(uchunk // 512 or 1):
                        w = min(512, uchunk)
                        nc.tensor.matmul(pu[:, j * w:(j + 1) * w], ubw[:],
                                         ur[:, j * w:(j + 1) * w], start=True, stop=True)
                    nc.scalar.activation(uhl[0:D, h * uchunk:(h + 1) * uchunk], pu[:],
                                         mybir.ActivationFunctionType.Copy)
                    nc.vector.tensor_sub(uhl[D:128, h * uchunk:(h + 1) * uchunk], pu[:],
                                         uhl[0:D, h * uchunk:(h + 1) * uchunk])

            # ---- phase C: per-tile ----
            with (
                tc.tile_pool(name="sc", bufs=2) as scp,
                tc.tile_pool(name="wk", bufs=2) as wk,
                tc.tile_pool(name="ps_sc", bufs=2, space="PSUM") as pssc,
                tc.tile_pool(name="ps_z", bufs=2, space="PSUM") as psz,
                tc.tile_pool(name="ps_tr", bufs=1, space="PSUM") as pstr,
                tc.tile_pool(name="ps_t1", bufs=1, space="PSUM") as pst1,
            ):
                for qt in range(ntiles * c_repeats):
                    qt = qt % ntiles
                    q0 = qt * 128
                    # C1: scores
                    ssb = scp.tile([128, ncand], F32, tag="ssb")
                    for h in range(ncand // 1024):
                        pst = pssc.tile([128, 1024], F32, tag="psc")
                        for j in range(2):
                            c0 = h * 1024 + j * 512
                            nc.tensor.matmul(pst[:, j * 512:(j + 1) * 512],
                                             sc_l[:, q0:q0 + 128],
                                             sc_r[:, c0:c0 + 512],
                                             start=True, stop=True)
                        nc.scalar.activation(ssb[:, h * 1024:(h + 1) * 1024], pst[:],
                                             mybir.ActivationFunctionType.Copy)
                    # C2: top16
                    v8a = wk.tile([128, 8], F32, tag="v8a")
                    v8b = wk.tile([128, 8], F32, tag="v8b")
                    i32 = wk.tile([128, 16], mybir.dt.uint32, tag="i32")
                    nc.vector.max(out=v8a[:], in_=ssb[:])
                    nc.vector.max_index(out=i32[:, 0:8], in_max=v8a[:], in_values=ssb[:])
                    nc.vector.match_replace(out=ssb[:], in_to_replace=v8a[:],
                                            in_values=ssb[:], imm_value=-1e30)
                    nc.vector.max(out=v8b[:], in_=ssb[:])
                    nc.vector.max_index(out=i32[:, 8:16], in_max=v8b[:], in_values=ssb[:])
                    # C3: idx -> fp32 -> replicate x8 in free dim -> PE transpose
                    i16f = wk.tile([128, 16], F32, tag="i16f")
                    nc.vector.tensor_copy(i16f[:], i32[:])
                    i16r = wk.tile([128, 128], F32, tag="i16r")
                    rep = i16f[:].unsqueeze(1).to_broadcast([128, 8, 16])
                    nc.vector.tensor_copy(i16r[:].rearrange("p (r k) -> p r k", k=16), rep)
                    ptr = pstr.tile([128, 128], F32, tag="ptr")
                    nc.tensor.transpose(ptr[:], i16r[:], idu[:])
                    idxs = wk.tile([128, 128], I16, tag="idxs")
                    nc.vector.tensor_copy(idxs[:], ptr[:])
                    # C4: gather v pairs -> [128, 2048] bf16 (cols q*16+k)
                    gt = wk.tile([128, 1, 2048], BF, tag="gt")
                    if use_gather:
                        nc.gpsimd.dma_gather(out_ap=gt[:], in_ap=vpair[:], idxs_ap=idxs[:],
                                             num_idxs=2048, num_idxs_reg=2048,
                                             elem_size=128, transpose=True,
                                             single_packet=False)
                    else:
                        nc.vector.memset(gt[:], 0.0)
                    gtf = gt[:].rearrange("p a n -> p (a n)")
                    # C5-C8 per 512-col block (32 queries)
                    # leaky(y) folded into next matmul: rhs = [relu(y); relu(-y)],
                    # lhsT = [W^T; -0.1 W^T].  Final leaky commutes with max-pool.
                    pooled = wk.tile([D, 128], F32, tag="pooled")
                    # materialize u broadcast (each query's u column repeated 16x)
                    urep = wk.tile([128, 2048], BF, tag="urep")
                    ub_b = uhl[:, q0:q0 + 128].unsqueeze(2).to_broadcast([128, 128, KNN])
                    nc.scalar.activation(urep[:].rearrange("p (q k) -> p q k", k=KNN),
                                         ub_b, mybir.ActivationFunctionType.Copy)
                    for cb in range(4):
                        cbase = cb * 512
                        pz0 = psz.tile([D, 512], F32, tag="pz")
                        nc.tensor.matmul(pz0[:], iis[:], gtf[:, cbase:cbase + 512],
                                         start=True, stop=False)
                        nc.tensor.matmul(pz0[:], iis[:], urep[:, cbase:cbase + 512],
                                         start=False, stop=True)
                        rp0 = wk.tile([2 * D, 512], F32R, tag="rp0")
                        nc.scalar.activation(rp0[0:D, :], pz0[:],
                                             mybir.ActivationFunctionType.Relu)
                        nc.scalar.activation(rp0[D:2 * D, :], pz0[:],
                                             mybir.ActivationFunctionType.Relu, scale=-1.0)
                        pz1 = psz.tile([D, 512], F32, tag="pz")
                        nc.tensor.matmul(pz1[:], w0s[:], rp0[:], start=True, stop=True)
                        rp1 = wk.tile([2 * D, 512], F32R, tag="rp0")
                        nc.scalar.activation(rp1[0:D, :], pz1[:],
                                             mybir.ActivationFunctionType.Relu, bias=b0s[:])
                        nc.scalar.activation(rp1[D:2 * D, :], pz1[:],
                                             mybir.ActivationFunctionType.Relu,
                                             bias=b0ns[:], scale=-1.0)
                        pz2 = psz.tile([D, 512], F32, tag="pz")
                        nc.tensor.matmul(pz2[:], w1s[:], rp1[:], start=True, stop=True)
                        # pool over k=16 straight from PSUM (pre-activation; leaky
                        # and +b1 are applied after pooling - both monotonic)
                        nc.vector.tensor_reduce(
                            out=pooled[:, cb * 32:(cb + 1) * 32],
                            in_=pz2[:].rearrange("p (q k) -> p q k", k=KNN),
                            axis=mybir.AxisListType.X, op=mybir.AluOpType.max)
                    # t-linear: lhsT = [relu(pooled+b1); relu(-pooled-b1)] (f32r),
                    # rhs = [tw^T; -0.1 tw^T]; bias via K=1 ones x tb matmul.
                    tl = wk.tile([2 * D, 128], F32R, tag="tl")
                    nc.scalar.activation(tl[0:D, :], pooled[:],
                                         mybir.ActivationFunctionType.Relu, bias=b1s[:])
                    nc.scalar.activation(tl[D:2 * D, :], pooled[:],
                                         mybir.ActivationFunctionType.Relu,
                                         bias=b1ns[:], scale=-1.0)
                    pt1 = pst1.tile([128, 128], F32, tag="pt1")
                    nc.tensor.matmul(pt1[:], tl[:], trs[:], start=True, stop=False)
                    nc.tensor.matmul(pt1[:], ones1[:], tbs[:], start=False, stop=True)
                    outt = wk.tile([128, 128], F32, tag="outt")
                    nc.scalar.activation(outt[:], pt1[:], mybir.ActivationFunctionType.Copy)
                    nc.sync.dma_start(out=out[q0:q0 + 128, :], in_=outt[:])

    nc.compile()
    return nc


def _split_bf16(x, n):
    parts = []
    rem = np.asarray(x, np.float64)
    for _ in range(n):
        p = rem.astype(BF16)
        parts.append(p)
        rem = rem - p.astype(np.float64)
    return parts


def prep_core_inputs(qxyz, qfeat, cxyz, cfeat, pos_w, pos_b, tw, tb):
    """Build the per-core input map. All host work is O(N*small) layout prep."""
    nq = qxyz.shape[0]
    ncand = cxyz.shape[0]
    A = _split_bf16(2.0 * qxyz, 3)           # each [nq, 3]
    P = _split_bf16(cxyz, 3)                 # each [ncand, 3]
    m = _split_bf16(-np.sum(cxyz.astype(np.float64) ** 2, -1), 3)

    # order products by (i+j) descending so small terms accumulate first
    rows_q = []
    rows_c = []
    prods = sorted(((i, j) for i in range(3) for j in range(3)),
                   key=lambda t: -(t[0] + t[1]))
    for (i, j) in prods:
        for c in range(3):
            rows_q.append(A[i][:, c])
            rows_c.append(P[j][:, c])
    ones = np.ones(nq, BF16)
    for t in (m[2], m[1], m[0]):
        rows_q.append(ones)
        rows_c.append(t)
    sc_lhsT = np.stack(rows_q).astype(BF16)      # [30, nq]
    sc_rhs = np.stack(rows_c).astype(BF16)       # [30, ncand]

    vb_lhsT = np.concatenate([cxyz.T, cfeat.T]).astype(np.float32)       # [67, ncand]
    vb_rhs = np.concatenate([pos_w.T, np.eye(D)]).astype(np.float32)     # [67, 64]
    ub_lhsT = np.concatenate([-pos_w.T, np.eye(D), pos_b[None, :]]).astype(np.float32)  # [68, 64]
    ub_rhs = np.concatenate([qxyz.T, qfeat.T, np.ones((1, nq))]).astype(np.float32)     # [68, nq]

    t_rhs = np.concatenate([tw.T, -LEAKY * tw.T]).astype(np.float32)     # [128, 128]
    tb_row = tb[None, :].astype(np.float32)
    ii = np.concatenate([np.eye(D), np.eye(D)]).astype(BF16)             # [128, 64]
    idu = np.eye(128).astype(np.float32)

    return {
        "sc_lhsT": sc_lhsT, "sc_rhs": sc_rhs,
        "vb_lhsT": vb_lhsT, "vb_rhs": vb_rhs,
        "ub_lhsT": ub_lhsT, "ub_rhs": ub_rhs,
        "w0T": None, "w1T": None,  # filled by caller (shared)
        "b0c": None, "b1c": None, "b0n": None, "b1n": None,
        "t_rhs": t_rhs, "tb_row": tb_row, "ii128": ii, "id128u": idu,
    }


def build_in_maps(inputs):
    pc1 = np.asarray(inputs["pc1"]); pc2 = np.asarray(inputs["pc2"])
    feat1 = np.asarray(inputs["feat1"]); feat2 = np.asarray(inputs["feat2"])
    pos_w = np.asarray(inputs["pos_w"]); pos_b = np.asarray(inputs["pos_b"])
    w0 = np.asarray(inputs["mlp_w0"]); b0 = np.asarray(inputs["mlp_b0"])
    w1 = np.asarray(inputs["mlp_w1"]); b1 = np.asarray(inputs["mlp_b1"])
    t1w = np.asarray(inputs["t1_w"]); t1b = np.asarray(inputs["t1_b"])
    t2w = np.asarray(inputs["t2_w"]); t2b = np.asarray(inputs["t2_b"])

    w0T = np.concatenate([w0.T, -LEAKY * w0.T]).astype(np.float32)
    w1T = np.concatenate([w1.T, -LEAKY * w1.T]).astype(np.float32)
    b0c = b0.astype(np.float32)[:, None].copy()
    b1c = b1.astype(np.float32)[:, None].copy()

    half = NQ_TOT
    in_maps = []
    core_meta = []
    for d in range(2):
        for b in range(2):
            for h in range(2):
                if d == 0:
                    q, p, fq, fp, tw, tb = pc1[b], pc2[b], feat1[b], feat2[b], t1w, t1b
                else:
                    q, p, fq, fp, tw, tb = pc2[b], pc1[b], feat2[b], feat1[b], t2w, t2b
                sl = slice(h * half, (h + 1) * half)
                m = prep_core_inputs(q[sl], fq[sl], p, fp, pos_w, pos_b, tw, tb)
                m["w0T"] = w0T; m["w1T"] = w1T; m["b0c"] = b0c; m["b1c"] = b1c
                m["b0n"] = -b0c; m["b1n"] = -b1c
                in_maps.append(m)
                core_meta.append((d, b, h))
    return in_maps, core_meta


def kernel(pc1, pc2, feat1, feat2, pos_w, pos_b, mlp_w0, mlp_b0,
           mlp_w1, mlp_b1, t1_w, t1_b, t2_w, t2_b, _trace=False):
    pc1 = np.asarray(pc1)

    if "nc" not in _CACHE:
        _CACHE["nc"] = build_nc()
    nc = _CACHE["nc"]

    inputs = dict(pc1=pc1, pc2=pc2, feat1=feat1, feat2=feat2, pos_w=pos_w,
                  pos_b=pos_b, mlp_w0=mlp_w0, mlp_b0=mlp_b0, mlp_w1=mlp_w1,
                  mlp_b1=mlp_b1, t1_w=t1_w, t1_b=t1_b, t2_w=t2_w, t2_b=t2_b)
    in_maps, core_meta = build_in_maps(inputs)
    _CACHE["last_in_maps"] = in_maps

    res = run_bass_kernel_spmd(nc, in_maps, core_ids=list(range(8)), trace=_trace)
    _CACHE["last_res"] = res
    half = NQ_TOT

    B, N = pc1.shape[0], pc1.shape[1]
    f1 = np.zeros((B, N, 128), np.float32)
    f2 = np.zeros((B, N, 128), np.float32)
    for (dd, b, h), r in zip(core_meta, res.results):
        o = r["out"]
        tgt = f1 if dd == 0 else f2
        tgt[b, h * half:(h + 1) * half, :] = o
    return f1, f2


if __name__ == "__main__":
    # quick smoke with random data
    rng = np.random.default_rng(0)
    B, N = 2, 8192
    ins = {
        "pc1": rng.standard_normal((B, N, 3), np.float32),
        "pc2": rng.standard_normal((B, N, 3), np.float32),
        "feat1": rng.standard_normal((B, N, D), np.float32),
        "feat2": rng.standard_normal((B, N, D), np.float32),
        "pos_w": (rng.standard_normal((D, 3)) * 0.1).astype(np.float32),
        "pos_b": (rng.standard_normal((D,)) * 0.1).astype(np.float32),
        "mlp_w0": (rng.standard_normal((D, D)) * 0.1).astype(np.float32),
        "mlp_b0": (rng.standard_normal((D,)) * 0.1).astype(np.float32),
        "mlp_w1": (rng.standard_normal((D, D)) * 0.1).astype(np.float32),
        "mlp_b1": (rng.standard_normal((D,)) * 0.1).astype(np.float32),
        "t1_w": (rng.standard_normal((128, D)) * 0.1).astype(np.float32),
        "t1_b": (rng.standard_normal((128,)) * 0.1).astype(np.float32),
        "t2_w": (rng.standard_normal((128, D)) * 0.1).astype(np.float32),
        "t2_b": (rng.standard_normal((128,)) * 0.1).astype(np.float32),
    }
    f1, f2 = kernel(**ins)
    print("f1", f1.shape, "f2", f2.shape)

